# revision 4
# baseline (speedup 1.0000x reference)
"""Trainium2 Bass kernel for ItemEmbeddingLayer (embedding_lookup).

Reference computation:
    out = Q_matrix[items] @ skill_embedding[user]      # [8192, 128] f32

Sharding: the active user's embedding row ([256,128]) is replicated to all
8 cores; `items` is sharded batch-wise, 1024 per core; Q_matrix (bf16,
exact: Q is 0/1) is replicated in DRAM and each core pulls only the rows
its items need.

Per-core device kernel (v2 — gather-transpose):
  1. NSPLIT dma_gather(transpose=True) instructions pull the needed Q rows
     straight into [skill, item] (lhsT) layout — one SWDGE instruction per
     512 rows (~1.2us each on GpSimd) instead of 8 indirect DMAs + 16 PE
     transposes + 16 PSUM copies.
  2. emb is pre-split on host into bf16 hi+lo parts packed side by side
     ([128, 2, 256]: hi in cols 0:128, lo in 128:256), recovering ~fp32
     precision from bf16 matmuls; rhs N=256 does hi and lo in one pass.
  3. 8 item-chunks x 2 skill-chunk matmuls accumulate in fp32 PSUM.
  4. One DVE add per chunk fuses hi+lo and moves PSUM -> SBUF.
  5. One 256KB DMA out per gather half.
"""

import numpy as np
import ml_dtypes

import concourse.bass as bass
import concourse.bacc as bacc
import concourse.mybir as mybir
from concourse.tile import TileContext
from concourse.bass_utils import run_bass_kernel_spmd

N_CORES = 8
L = 8192            # total items (seq len)
LC = L // N_CORES   # items per core
S = 256             # skills
K = 128             # hidden
R = 4096            # Q_matrix rows (item vocab)
P = 128             # partitions
NSPLIT = 2          # gathers per core (pipeline gather vs matmul)
GL = LC // NSPLIT   # items per gather
NCH = LC // P       # 128-item chunks per core


def build_bass() -> bass.Bass:
    nc = bacc.Bacc(trn_type="TRN2", dynamic_dma_scratch_size=131072)
    q = nc.declare_dram_parameter("q_bf16", [R, S], mybir.dt.bfloat16, isOutput=False)
    idx = nc.declare_dram_parameter("idx", [P, LC // 16], mybir.dt.int16, isOutput=False)
    emb = nc.declare_dram_parameter("emb", [P, 2, 2 * K], mybir.dt.bfloat16, isOutput=False)
    out = nc.declare_dram_parameter("out", [LC, K], mybir.dt.float32, isOutput=True)

    out_r = out[:].rearrange("(c p) k -> p c k", p=P)  # [128, NCH, 128]

    with (
        TileContext(nc) as tc,
        tc.tile_pool(name="main", bufs=1) as pool,
        tc.tile_pool(name="acc", bufs=4, space="PSUM") as apsum,
    ):
        idx_t = pool.tile([P, LC // 16], mybir.dt.int16)
        nc.sync.dma_start(out=idx_t[:], in_=idx[:])
        emb_t = pool.tile([P, 2, 2 * K], mybir.dt.bfloat16)
        nc.sync.dma_start(out=emb_t[:], in_=emb[:])

        obuf = pool.tile([P, NCH, K], mybir.dt.float32)
        qT = [
            pool.tile([P, 2, GL], mybir.dt.bfloat16, name=f"qT{h}")
            for h in range(NSPLIT)
        ]
        for h in range(NSPLIT):
            # qT[h][p, j, i] = Q[idx[h*GL + i], j*128 + p]  (lhsT layout)
            nc.gpsimd.dma_gather(
                qT[h][:],
                q[:],
                idx_t[:, h * (GL // 16) : (h + 1) * (GL // 16)],
                GL,
                GL,
                S,
                transpose=True,
            )
            for m in range(GL // P):
                c = h * (GL // P) + m
                # ps = q@hi + q@lo, all four (skill-chunk, hi/lo) partial
                # products accumulate in fp32 PSUM
                ps = apsum.tile([P, K], mybir.dt.float32, tag="ps")
                for e in range(4):
                    j, part = e % 2, e // 2
                    nc.tensor.matmul(
                        ps[:],
                        qT[h][:, j, m * P : (m + 1) * P],
                        emb_t[:, j, part * K : (part + 1) * K],
                        start=(e == 0),
                        stop=(e == 3),
                    )
                nc.vector.tensor_copy(obuf[:, c, :], ps[:])
            nc.sync.dma_start(
                out=out_r[:, h * (GL // P) : (h + 1) * (GL // P), :],
                in_=obuf[:, h * (GL // P) : (h + 1) * (GL // P), :],
            )

    nc.compile()
    return nc


_CACHE: dict = {}


def get_nc() -> bass.Bass:
    if "nc" not in _CACHE:
        _CACHE["nc"] = build_bass()
    return _CACHE["nc"]


def make_in_maps(user, Q_matrix, items, skill_embedding):
    user = int(np.asarray(user))
    Q = np.asarray(Q_matrix, dtype=np.float32)
    items = np.asarray(items).astype(np.int64)
    emb32 = np.ascontiguousarray(np.asarray(skill_embedding)[user], dtype=np.float32)
    q_bf = Q.astype(ml_dtypes.bfloat16)  # exact: Q is 0/1

    # emb = hi + lo, both bf16; [128, 2, 256]: [s%128, s//128, hi k | lo k]
    hi = emb32.astype(ml_dtypes.bfloat16)
    lo = (emb32 - hi.astype(np.float32)).astype(ml_dtypes.bfloat16)
    emb_hilo = np.concatenate(
        [hi.reshape(2, P, K), lo.reshape(2, P, K)], axis=2
    ).transpose(1, 0, 2)  # [128, 2, 256]
    emb_hilo = np.ascontiguousarray(emb_hilo)

    in_maps = []
    for i in range(N_CORES):
        it = items[i * LC : (i + 1) * LC].astype(np.int16)
        # dma_gather idx i lives at [i % 16, i // 16], replicated to all
        # 8 groups of 16 partitions (one per GpSimd DSP core).
        blk = np.ascontiguousarray(it.reshape(LC // 16, 16).T)  # [16, LC//16]
        idx_arr = np.tile(blk, (8, 1))  # [128, LC//16]
        in_maps.append({"q_bf16": q_bf, "idx": idx_arr, "emb": emb_hilo})
    return in_maps


def kernel(user, Q_matrix, items, skill_embedding, _trace=False, _result_box=None):
    in_maps = make_in_maps(user, Q_matrix, items, skill_embedding)
    res = run_bass_kernel_spmd(get_nc(), in_maps, list(range(N_CORES)), trace=_trace)
    if _result_box is not None:
        _result_box.append(res)
    out = np.concatenate([res.results[i]["out"] for i in range(N_CORES)], axis=0)
    return np.ascontiguousarray(out, dtype=np.float32)
